# revision 1
# baseline (speedup 1.0000x reference)
"""BoneLinear Trainium2 kernel (8-core SPMD, data-parallel over batch).

Math: reference computes out = x @ (weight + w)^T where w is the bone
block-update of weight:
    wblk = weight.reshape(a, r, b, r).transpose(0,2,1,3)      # (a,b,r,r)
    wup  = wblk @ bone[b] + bone[b]                            # per (a,b)
    w    = wup.transpose(0,2,1,3).reshape(out_f, in_f)

Identity used here (verified numerically): with y[:, b*r:(b+1)*r] =
x[:, b*r:(b+1)*r] @ bone[b].T and s = sum_b y[:, b-block]:

    out = (x + y) @ weight^T + tile(s over out-blocks)

so the heavy GEMM uses the *original* weight; the bone update reduces to a
cheap block-diagonal transform of x plus a rank-64 broadcast correction.

Per core (batch element): z^T = x^T + blockdiag(bone^T) @ x^T is computed on
the PE in t-quarters and kept SBUF-resident in fp16; the main GEMM
out = z^T.T @ W^T streams W^T from HBM; s is accumulated on the PE and added
(broadcast over 64-column blocks) during PSUM eviction.
"""

import numpy as np

B, T, IN, OUT, R = 8, 2048, 4096, 4096, 64
P = 128
KT = IN // P  # 32 contraction tiles
TQ = 512  # t-quarter size
NQ = T // TQ  # 4 quarters
NFREE = 512  # matmul moving free dim / o-tile size
OTN = OUT // NFREE  # 8 o-tiles

_NC_CACHE = {}


def _build_nc(
    reps=1,
    nfree=NFREE,
    po_bufs=4,
    py_bufs=2,
    xt_bufs=36,
    wt_on_act=True,
    act_copy=True,
    xt_chunk=1,
    interleave_p1=True,
    fuse_start=True,
):
    import concourse.mybir as mybir
    from concourse import bacc
    from concourse.tile import TileContext
    from concourse.masks import make_identity

    F16 = mybir.dt.float16
    F32 = mybir.dt.float32
    otn = OUT // nfree

    nc = bacc.Bacc(None, target_bir_lowering=False)
    xT = nc.dram_tensor("xT", [IN, T], F16, kind="ExternalInput")
    wT = nc.dram_tensor("wT", [IN, OUT], F16, kind="ExternalInput")
    bd = nc.dram_tensor("bd", [P, KT, P], F16, kind="ExternalInput")
    bdv = nc.dram_tensor("bdv", [P, KT, R], F16, kind="ExternalInput")
    out = nc.dram_tensor("out", [T, OUT], F32, kind="ExternalOutput")

    wTv = wT.rearrange("(ko p) o -> p ko o", p=P)

    with TileContext(nc) as tc:
        with (
            tc.tile_pool(name="const", bufs=1) as constp,
            tc.tile_pool(name="xt", bufs=xt_bufs) as xpool,
            tc.tile_pool(name="zt", bufs=2) as zpool,
            tc.tile_pool(name="wt", bufs=2) as wpool,
            tc.tile_pool(name="sb", bufs=2) as spool,
            tc.tile_pool(name="ob", bufs=3) as opool,
            tc.tile_pool(name="py", bufs=py_bufs, space="PSUM") as pyp,
            tc.tile_pool(name="ps", bufs=1, space="PSUM") as psp,
            tc.tile_pool(name="po", bufs=po_bufs, space="PSUM") as pop,
        ):
            bd_sb = constp.tile([P, KT, P], F16, tag="bd")
            nc.sync.dma_start(bd_sb[:], bd[:])
            bdv_sb = constp.tile([P, KT, R], F16, tag="bdv")
            nc.sync.dma_start(bdv_sb[:], bdv[:])
            ident = constp.tile([R, R], F32, tag="ident")
            make_identity(nc, ident)

            # Next-quarter xt tiles are prefetched during the current
            # quarter's phase 2, so phase 1 never waits on DMA (and the small
            # xt loads don't get stuck behind a 4MB wt transfer at the
            # quarter boundary). xt_chunk>1 batches that many k-tiles per
            # DMA (fewer, larger transfers).
            xt_tiles = {}
            xTv = xT.rearrange("(ko p) t -> p ko t", p=P)

            def prefetch_xt(qi):
                tq0 = (qi % NQ) * TQ
                tiles = []
                for kc in range(0, KT, xt_chunk):
                    xt = xpool.tile([P, xt_chunk, TQ], F16, tag="xt")
                    nc.sync.dma_start(
                        xt[:], xTv[:, kc : kc + xt_chunk, tq0 : tq0 + TQ]
                    )
                    for j in range(xt_chunk):
                        tiles.append(xt[:, j, :])
                xt_tiles[qi] = tiles

            def build_phase1(qi, nchunks):
                """Allocate quarter-qi phase-1 tiles. Returns (zt, s_sb, emit)
                where emit(ci) emits chunk ci of the k-loop; the last chunk
                also emits the s finalize (fold + transpose to t-layout)."""
                zt = zpool.tile([P, KT, TQ], F16, tag="zt")
                s_sb = spool.tile([P, TQ // P, R], F32, tag="ssb")
                psum_s = psp.tile([R, TQ], F32, tag="ps")
                xts = xt_tiles.pop(qi)
                per = KT // nchunks

                def emit(ci):
                    for k in range(ci * per, (ci + 1) * per):
                        xt = xts[k]
                        py = pyp.tile([P, TQ], F32, tag="py")
                        # z^T tile directly: blockdiag(I + bone[2k]^T, ...)
                        # @ x^T (x rides the identity exactly — same rounding
                        # as an explicit fp32 add of fp16 x).
                        nc.tensor.matmul(
                            py[:], bd_sb[:, k, :], xt[:], start=True, stop=True
                        )
                        # s^T accumulation: vstack(bone[2k]^T, bone[2k+1]^T)
                        nc.tensor.matmul(
                            psum_s[:],
                            bdv_sb[:, k, :],
                            xt[:],
                            start=(k == 0),
                            stop=(k == KT - 1),
                        )
                        # psum -> SBUF fp16; split 2:1 across DVE and the
                        # idle ACT engine so copies keep pace with the PE.
                        if act_copy and k % 3 == 2:
                            nc.scalar.copy(zt[:, k, :], py[:])
                        else:
                            nc.vector.tensor_copy(zt[:, k, :], py[:])
                    if ci == nchunks - 1:
                        # s: [R, TQ] -> t-partition layout [P, TQ//P, R]
                        sT = spool.tile([R, TQ], F32, tag="sT")
                        nc.vector.tensor_copy(sT[:], psum_s[:])
                        for c in range(TQ // P):
                            pt = pyp.tile([P, R], F32, tag="py")
                            nc.tensor.transpose(
                                pt[:], sT[:, c * P : (c + 1) * P], ident[:]
                            )
                            nc.vector.tensor_copy(s_sb[:, c, :], pt[:])

                return zt, s_sb, emit

            # wt DMAs ride the ACT HWDGE ring (wt_on_act) so the next
            # quarter's first weight tile isn't FIFO-queued behind the 32
            # xt loads on the SP ring — hides the 4MB load under compute.
            wt_dma = nc.scalar.dma_start if wt_on_act else nc.sync.dma_start

            # reps>1 repeats the whole computation (timing builds only —
            # wall-time differencing cancels host/transfer overhead).
            prefetch_xt(0)
            if fuse_start:
                # Quarter 0's phase 1 is emitted k-by-k inside its first
                # o-tile (see below) so the PE stream stays dense from the
                # start instead of idling through a DMA-paced prologue.
                cur = build_phase1(0, KT)
            else:
                cur = build_phase1(0, 1)
                cur[2](0)
            for qi in range(NQ * reps):
                q = qi % NQ
                t0 = q * TQ
                zt, s_sb, _ = cur
                nxt = None
                # ---- phase 2: out quarter = z^T.T @ W^T + s ----
                # Phase 1 of quarter qi+1 is emitted in chunks between this
                # quarter's o-tiles (interleave_p1), so its short matmuls and
                # psum->SBUF copies hide inside the dense GEMM stream instead
                # of forming a serial DVE/ACT-paced wall at the boundary.
                for ot in range(otn):
                    wt = wpool.tile([P, KT, nfree], F16, tag="wt")
                    if qi == 0 and ot == 0 and fuse_start:
                        # Chunked so the fused k-loop's first matmuls don't
                        # gate on the full 4MB transfer.
                        for kc in range(0, KT, 8):
                            wt_dma(
                                wt[:, kc : kc + 8, :],
                                wTv[:, kc : kc + 8, ot * nfree : (ot + 1) * nfree],
                            )
                    else:
                        wt_dma(wt[:], wTv[:, :, ot * nfree : (ot + 1) * nfree])
                    if ot == 0 and qi + 1 < NQ * reps:
                        # Next quarter's x loads: emitted after this quarter's
                        # first weight tile so the SP ring serves wt first.
                        prefetch_xt(qi + 1)
                        if interleave_p1 == 2:
                            nxt = build_phase1(qi + 1, otn * (TQ // P))
                        elif interleave_p1:
                            nxt = build_phase1(qi + 1, otn)
                    if qi == 0 and ot == 0 and fuse_start:
                        # k-outer / tt-inner: emit phase-1 step k, then the
                        # four psum-group matmuls that consume zt[:, k].
                        pos = []
                        for _tt in range(TQ // P):
                            po_f = pop.tile([P, nfree], F32, tag="po")
                            pos.append(po_f)
                        for k in range(KT):
                            cur[2](k)
                            for tt in range(TQ // P):
                                nc.tensor.matmul(
                                    pos[tt][:],
                                    zt[:, k, tt * P : (tt + 1) * P],
                                    wt[:, k, :],
                                    start=(k == 0),
                                    stop=(k == KT - 1),
                                )
                        for tt in range(TQ // P):
                            ob = opool.tile([P, nfree], F32, tag="ob")
                            ob3 = ob.rearrange("p (a r) -> p a r", r=R)
                            po3 = pos[tt].rearrange("p (a r) -> p a r", r=R)
                            s_bcast = s_sb[:, tt, :][:, None, :].to_broadcast(
                                (P, nfree // R, R)
                            )
                            nc.vector.tensor_add(ob3, po3, s_bcast)
                            nc.sync.dma_start(
                                out[
                                    t0 + tt * P : t0 + (tt + 1) * P,
                                    ot * nfree : (ot + 1) * nfree,
                                ],
                                ob[:],
                            )
                        if nxt is not None and interleave_p1 != 2:
                            nxt[2](ot)
                        continue
                    for tt in range(TQ // P):
                        po = pop.tile([P, nfree], F32, tag="po")
                        for k in range(KT):
                            nc.tensor.matmul(
                                po[:],
                                zt[:, k, tt * P : (tt + 1) * P],
                                wt[:, k, :],
                                start=(k == 0),
                                stop=(k == KT - 1),
                            )
                        ob = opool.tile([P, nfree], F32, tag="ob")
                        ob3 = ob.rearrange("p (a r) -> p a r", r=R)
                        po3 = po.rearrange("p (a r) -> p a r", r=R)
                        s_bcast = s_sb[:, tt, :][:, None, :].to_broadcast(
                            (P, nfree // R, R)
                        )
                        nc.vector.tensor_add(ob3, po3, s_bcast)
                        nc.sync.dma_start(
                            out[
                                t0 + tt * P : t0 + (tt + 1) * P,
                                ot * nfree : (ot + 1) * nfree,
                            ],
                            ob[:],
                        )
                        if nxt is not None and interleave_p1 == 2:
                            nxt[2](ot * (TQ // P) + tt)
                    if nxt is not None and interleave_p1 != 2:
                        nxt[2](ot)
                if qi + 1 < NQ * reps and not interleave_p1:
                    nxt = build_phase1(qi + 1, 1)
                    nxt[2](0)
                cur = nxt
    nc.compile()
    return nc


def _get_nc(reps=1):
    key = ("nc", reps)
    if key not in _NC_CACHE:
        _NC_CACHE[key] = _build_nc(reps)
    return _NC_CACHE[key]


def prep_in_maps(x, weight, bone):
    """Host-side layout prep: transposes + block placement + fp16 cast."""
    x = np.asarray(x, dtype=np.float32)
    weight = np.asarray(weight, dtype=np.float32)
    bone = np.asarray(bone, dtype=np.float32)
    assert x.shape == (B, T, IN), x.shape
    assert weight.shape == (OUT, IN), weight.shape
    assert bone.shape == (IN // R, R, R), bone.shape

    wT16 = np.ascontiguousarray(weight.T).astype(np.float16)
    boneT = bone.transpose(0, 2, 1).astype(np.float16)  # bone[b]^T
    bdmat = np.zeros((KT, P, P), np.float16)
    bdmat[:, 0:R, 0:R] = boneT[0::2]
    bdmat[:, R:P, R:P] = boneT[1::2]
    bdmat += np.eye(P, dtype=np.float16)[None]  # fold the +x into the y-mm
    bd_host = np.ascontiguousarray(bdmat.transpose(1, 0, 2))  # [P, KT, P]
    bdvm = np.zeros((KT, P, R), np.float16)
    bdvm[:, 0:R, :] = boneT[0::2]
    bdvm[:, R:P, :] = boneT[1::2]
    bdv_host = np.ascontiguousarray(bdvm.transpose(1, 0, 2))  # [P, KT, R]

    in_maps = []
    for i in range(B):
        xT16 = np.ascontiguousarray(x[i].T).astype(np.float16)
        in_maps.append({"xT": xT16, "wT": wT16, "bd": bd_host, "bdv": bdv_host})
    return in_maps


def kernel(x, weight, bone):
    from concourse.bass_utils import run_bass_kernel_spmd

    nc = _get_nc()
    in_maps = prep_in_maps(x, weight, bone)
    res = run_bass_kernel_spmd(nc, in_maps, core_ids=list(range(B)))
    return np.stack([r["out"] for r in res.results], axis=0)


if __name__ == "__main__":
    rng = np.random.default_rng(0)
    x = rng.standard_normal((B, T, IN), dtype=np.float32)
    weight = (rng.standard_normal((OUT, IN)) * 0.02).astype(np.float32)
    bone = (rng.standard_normal((IN // R, R, R)) * 0.02).astype(np.float32)
    out = kernel(x=x, weight=weight, bone=bone)
    print(out.shape, out.dtype)



# revision 3
# speedup vs baseline: 1.2847x; 1.2847x over previous
"""BoneLinear Trainium2 kernel — fp8(DoubleRow)/fp16 hybrid, 8-core SPMD.

Math identity (as baseline): with z = x + y, y[:,b] = x[:,b] @ bone[b]^T,
s = sum_b y[:,b-block]:
    out = z @ W^T + tile(s over 64-wide out-blocks)

The heavy GEMM z @ W^T runs with the contraction dim split: the first K8
128-deep k-tiles use fp8e4 operands in DoubleRow mode (256-contraction per
instruction, ~1.4x fp16 throughput), the remaining 32-K8 k-tiles use fp16.
Both operand sets are pre-scaled by the same powers of two (z by 32, W by
1024) so they accumulate into one PSUM group at scale 32768; the eviction
rescales and adds s in one fused DVE op.

Orientation: W^T tiles are the stationary operand (streamed from HBM once),
z^T is moving (SBUF-resident for the whole T=2048), output is produced as
out^T [OUT, T] and transposed on host. Each stationary tile is reused for 4
consecutive matmuls (the four t-quarters).
"""

import numpy as np

B, T, IN, OUT, R = 8, 2048, 4096, 4096, 64
P = 128
KT = IN // P  # 32 contraction k-tiles
QT = 512  # t-quarter (moving free dim)
NQ = T // QT  # 4
OSL = OUT // P  # 32 output slices
K8 = 14  # k-tiles on the fp8 path (even); rest fp16. CPU-sim rel err:
# K8=16 -> 1.89e-2, K8=14 -> 1.77e-2, K8=12 -> 1.64e-2 (gate 2e-2)
ZSC = 32.0
WSC = 1024.0

_NC_CACHE = {}


def _build_nc(
    reps=1,
    k8=K8,
    xt_chunk=4,
    xt_bufs=6,
    wt_bufs=2,
    ob_bufs=4,
    py_bufs=2,
    ps_bufs=1,
    po_bufs=5,
    act_frac=3,
    out16=True,
):
    import concourse.mybir as mybir
    from concourse import bacc
    from concourse.tile import TileContext

    F8 = mybir.dt.float8e4
    F16 = mybir.dt.float16
    F32 = mybir.dt.float32
    DR = mybir.MatmulPerfMode.DoubleRow
    k16 = KT - k8
    kk8 = k8 // 2

    nc = bacc.Bacc(None, target_bir_lowering=False)
    xT = nc.dram_tensor("xT", [IN, T], F16, kind="ExternalInput")
    if k8:
        w8 = nc.dram_tensor("w8", [P, OSL, kk8, 2, P], F8, kind="ExternalInput")
    if k16:
        w16 = nc.dram_tensor("w16", [P, OSL, k16, P], F16, kind="ExternalInput")
    bd = nc.dram_tensor("bd", [P, KT, P], F16, kind="ExternalInput")
    bdv = nc.dram_tensor("bdv", [P, KT, P], F16, kind="ExternalInput")
    FOUT = F16 if out16 else F32
    outT = nc.dram_tensor("outT", [OUT, T], FOUT, kind="ExternalOutput")

    xTv = xT.rearrange("(ko p) t -> p ko t", p=P)

    with TileContext(nc) as tc:
        with (
            tc.tile_pool(name="const", bufs=1) as constp,
            tc.tile_pool(name="xt", bufs=xt_bufs) as xpool,
            tc.tile_pool(name="wt", bufs=wt_bufs) as wpool,
            tc.tile_pool(name="ob", bufs=ob_bufs) as opool,
            tc.tile_pool(name="py", bufs=py_bufs, space="PSUM") as pyp,
            tc.tile_pool(name="ps", bufs=ps_bufs, space="PSUM") as psp,
            tc.tile_pool(name="po", bufs=po_bufs, space="PSUM") as pop,
        ):
            bd_sb = constp.tile([P, KT, P], F16, tag="bd")
            nc.sync.dma_start(bd_sb[:], bd[:])
            bdv_sb = constp.tile([P, KT, P], F16, tag="bdv")
            nc.sync.dma_start(bdv_sb[:], bdv[:])

            nchunks = KT // xt_chunk
            xt_tiles = {}
            xt_rings = [nc.sync.dma_start, nc.gpsimd.dma_start]

            def prefetch_xt(qi):
                q = qi % NQ
                tq0 = q * QT
                tiles = []
                for ci in range(nchunks):
                    xt = xpool.tile([P, xt_chunk, QT], F16, tag="xt", name=f"xt_{qi}_{ci}")
                    xt_rings[ci % len(xt_rings)](
                        xt[:],
                        xTv[:, ci * xt_chunk : (ci + 1) * xt_chunk, tq0 : tq0 + QT],
                    )
                    for j in range(xt_chunk):
                        tiles.append(xt[:, j, :])
                xt_tiles[qi] = tiles

            prefetch_xt(0)
            for rep in range(reps):
                if k8:
                    zt8 = constp.tile([P, kk8, 2, T], F8, tag="zt8")
                if k16:
                    zt16 = constp.tile([P, k16, T], F16, tag="zt16")
                sdup = constp.tile([P, T], F32, tag="sdup")

                # weight-slice prefetch (ACT HWDGE ring; depth = wt_bufs)
                wts = {}

                def load_wt(s):
                    ent = []
                    if k8:
                        wt8 = wpool.tile([P, kk8, 2, P], F8, tag="wt8")
                        nc.scalar.dma_start(wt8[:], w8[:, s, :, :, :])
                        ent.append(wt8)
                    else:
                        ent.append(None)
                    if k16:
                        wt16 = wpool.tile([P, k16, P], F16, tag="wt16")
                        nc.scalar.dma_start(wt16[:], w16[:, s, :, :])
                        ent.append(wt16)
                    else:
                        ent.append(None)
                    wts[s] = ent

                for s in range(min(wt_bufs, OSL)):
                    load_wt(s)

                # ---- phase 1: z = (32x + 32Bx), s^T (duplicated) ----
                for q in range(NQ):
                    qi = rep * NQ + q
                    if qi + 1 < reps * NQ:
                        prefetch_xt(qi + 1)
                    tq0 = q * QT
                    xts = xt_tiles.pop(qi)
                    ps = psp.tile([P, QT], F32, tag="ps")
                    for k in range(KT):
                        xt = xts[k]
                        py = pyp.tile([P, QT], F32, tag="py")
                        nc.tensor.matmul(
                            py[:], bd_sb[:, k, :], xt, start=True, stop=True
                        )
                        nc.tensor.matmul(
                            ps[:],
                            bdv_sb[:, k, :],
                            xt,
                            start=(k == 0),
                            stop=(k == KT - 1),
                        )
                        if k < k8:
                            dst = zt8[:, k // 2, k % 2, tq0 : tq0 + QT]
                        else:
                            dst = zt16[:, k - k8, tq0 : tq0 + QT]
                        if k % act_frac == act_frac - 1:
                            nc.scalar.copy(dst, py[:])
                        else:
                            nc.vector.tensor_copy(dst, py[:])
                    nc.vector.tensor_copy(sdup[:, tq0 : tq0 + QT], ps[:])

                # ---- phase 2: out^T slice = (W-slice)^T z + s ----
                for s in range(OSL):
                    if s + wt_bufs < OSL:
                        load_wt(s + wt_bufs)
                    wt8, wt16 = wts.pop(s)
                    pos = [
                        pop.tile([P, QT], F32, tag="po", name=f"po_{s}_{q}")
                        for q in range(NQ)
                    ]
                    for kk in range(kk8):
                        for q in range(NQ):
                            nc.tensor.matmul(
                                pos[q][:],
                                wt8[:, kk, :, :],
                                zt8[:, kk, :, q * QT : (q + 1) * QT],
                                start=(kk == 0),
                                stop=(k16 == 0 and kk == kk8 - 1),
                                perf_mode=DR,
                            )
                    for j in range(k16):
                        for q in range(NQ):
                            nc.tensor.matmul(
                                pos[q][:],
                                wt16[:, j, :],
                                zt16[:, j, q * QT : (q + 1) * QT],
                                start=(k8 == 0 and j == 0),
                                stop=(j == k16 - 1),
                            )
                    for q in range(NQ):
                        ob = opool.tile([P, QT], FOUT, tag="ob", name=f"ob_{s}_{q}")
                        nc.vector.scalar_tensor_tensor(
                            ob[:],
                            pos[q][:],
                            1.0 / (ZSC * WSC),
                            sdup[:, q * QT : (q + 1) * QT],
                            mybir.AluOpType.mult,
                            mybir.AluOpType.add,
                        )
                        nc.sync.dma_start(
                            outT[s * P : (s + 1) * P, q * QT : (q + 1) * QT], ob[:]
                        )
    nc.compile()
    return nc


def _get_nc(reps=1, **kw):
    key = ("nc", reps, tuple(sorted(kw.items())))
    if key not in _NC_CACHE:
        _NC_CACHE[key] = _build_nc(reps, **kw)
    return _NC_CACHE[key]


def prep_in_maps(x, weight, bone, k8=K8):
    import ml_dtypes

    F8NP = ml_dtypes.float8_e4m3
    x = np.asarray(x, dtype=np.float32)
    weight = np.asarray(weight, dtype=np.float32)
    bone = np.asarray(bone, dtype=np.float32)
    assert x.shape == (B, T, IN)
    assert weight.shape == (OUT, IN)
    assert bone.shape == (IN // R, R, R)
    k16 = KT - k8
    kk8 = k8 // 2
    k8e = k8 * P

    wT = np.ascontiguousarray(weight.T)  # [IN, OUT]
    shared = {}
    if k8:
        w8v = (WSC * wT[:k8e]).reshape(kk8, 2, P, OSL, P)
        shared["w8"] = np.ascontiguousarray(w8v.transpose(2, 3, 0, 1, 4)).astype(F8NP)
    if k16:
        w16v = (WSC * wT[k8e:]).reshape(k16, P, OSL, P)
        shared["w16"] = np.ascontiguousarray(w16v.transpose(1, 2, 0, 3)).astype(
            np.float16
        )

    boneT = bone.transpose(0, 2, 1)
    bdmat = np.zeros((KT, P, P), np.float32)
    bdmat[:, 0:R, 0:R] = boneT[0::2]
    bdmat[:, R:P, R:P] = boneT[1::2]
    bdmat += np.eye(P, dtype=np.float32)[None]
    bdmat *= ZSC
    shared["bd"] = np.ascontiguousarray(bdmat.transpose(1, 0, 2)).astype(np.float16)

    bdvm = np.zeros((KT, P, P), np.float32)
    bdvm[:, 0:R, 0:R] = boneT[0::2]
    bdvm[:, R:P, 0:R] = boneT[1::2]
    bdvm[:, :, R:P] = bdvm[:, :, 0:R]
    shared["bdv"] = np.ascontiguousarray(bdvm.transpose(1, 0, 2)).astype(np.float16)

    in_maps = []
    for i in range(B):
        xT16 = np.ascontiguousarray(x[i].T).astype(np.float16)
        in_maps.append({"xT": xT16, **shared})
    return in_maps


def kernel(x, weight, bone):
    from concourse.bass_utils import run_bass_kernel_spmd

    nc = _get_nc()
    in_maps = prep_in_maps(x, weight, bone)
    res = run_bass_kernel_spmd(nc, in_maps, core_ids=list(range(B)))
    return np.stack(
        [r["outT"].T.astype(np.float32) for r in res.results], axis=0
    )


if __name__ == "__main__":
    rng = np.random.default_rng(0)
    x = rng.standard_normal((B, T, IN), dtype=np.float32)
    weight = (rng.standard_normal((OUT, IN)) * 0.02).astype(np.float32)
    bone = (rng.standard_normal((IN // R, R, R)) * 0.02).astype(np.float32)
    out = kernel(x=x, weight=weight, bone=bone)
    print(out.shape, out.dtype)
